# revision 32
# baseline (speedup 1.0000x reference)
"""Trainium2 Bass kernel for nn_HGNN_ATT (HGNN message passing, K sub-graphs).

Sharding: nodes re-permuted so every shard holds an equal mix of users and
items (shard m = users [6250m..) ++ items [6250m..)), padded to 12544 rows.
Each directed edge is owned by the core owning its accumulation target.

spmm datapath (fp16): AllGather h (fp16 rows padded to 128 cols so each
gather descriptor is 256B) -> dma_gather chunks -> DVE scale by val ->
one-hot S per 128-edge tile (is_equal vs iota, fp16) -> PE fp16 matmul into
a 2KB PSUM bank holding one window-octet (8 windows x 64 cols).  Edges are
sorted (octet, bucket, window) so an octet's tiles across all buckets form
ONE PSUM accumulation group (start=True zeroes the whole 2KB zero region on
the octet's first tile; stop=True only on its last) -> one ACT copy per
octet writes PSUM to the SBUF acc.  Softmax/fc1/fusion-gate run in
feature-major fp32 layout as before.
"""

import numpy as np

import concourse.bass as bass
import concourse.mybir as mybir
import concourse.tile as tile
from concourse import bacc
from concourse.masks import make_identity

F32 = mybir.dt.float32
F16 = mybir.dt.float16
I16 = mybir.dt.int16
I32 = mybir.dt.int32
AF = mybir.ActivationFunctionType

NCORES = 8
D = 64
P = 128
TW = 128                # padded table row width (TW*2B = 256B descriptors)
BUCKET = 32768          # int16 gather index range
B_S = 16                # tiles per S-build DVE instruction
CCH = 512               # post-phase column chunk (PSUM free-dim limit)


OCT = 8                 # windows per PSUM zero region (2KB bank / 256B)


class Cfg:
    def __init__(self, NU, NI, K, E, CH):
        assert NU % NCORES == 0 and NI % NCORES == 0
        self.NU, self.NI, self.K, self.E, self.CH = NU, NI, K, E, CH
        self.UPC = NU // NCORES
        self.IPC = NI // NCORES
        self.SH = self.UPC + self.IPC            # real rows per shard
        self.SHP = ((self.SH + P - 1) // P) * P  # padded rows per shard
        self.NT = self.SHP // P                  # target windows per shard
        self.GN = NCORES * self.SHP              # padded global table rows
        self.NO = (self.NT + OCT - 1) // OCT     # window octets
        # buckets = local-window quarters, so each bucket's table slice is
        # filled by its own (small) AllGather and gathers overlap collectives
        self.NB = min(4, self.NT)
        wq = [self.NT // self.NB + (1 if i < self.NT % self.NB else 0)
              for i in range(self.NB)]
        self.QW = wq                                  # windows per quarter
        self.QR = [w * P for w in wq]                 # local rows per quarter
        self.QRS = np.concatenate([[0], np.cumsum(self.QR)]).astype(np.int64)
        self.BK = [NCORES * r for r in self.QR]       # bucket table rows
        self.BKS = np.concatenate([[0], np.cumsum(self.BK)]).astype(np.int64)
        assert max(self.BK) <= 32768                  # int16 gather indices


def _perm_maps(cfg):
    """original node id -> (core, local_row, padded_global_row).

    Global gather rows are quarter-major: bucket b holds every core's
    quarter-b rows (core-major inside the bucket)."""
    orig = np.arange(cfg.NU + cfg.NI)
    is_item = orig >= cfg.NU
    core = np.where(is_item, (orig - cfg.NU) // cfg.IPC, orig // cfg.UPC)
    loc = np.where(is_item, cfg.UPC + (orig - cfg.NU) % cfg.IPC, orig % cfg.UPC)
    q = np.searchsorted(cfg.QRS, loc, side="right") - 1
    g = cfg.BKS[q] + core * np.asarray(cfg.QR)[q] + (loc - cfg.QRS[q])
    return core.astype(np.int64), loc.astype(np.int64), g.astype(np.int64)


def prep(cfg, rows, cols):
    """Host-side graph preprocessing.

    Returns (plan, per-core arrays).  plan[k] holds per-(octet, bucket)
    chunk lists and the per-tile (w_local, start, stop) schedule shared by
    both spmm phases of sub-graph k.
    """
    NU, NI, K, CH = cfg.NU, cfg.NI, cfg.K, cfg.CH
    N = NU + NI
    NB, NT, NO = cfg.NB, cfg.NT, cfg.NO
    core_of, loc_of, g_of = _perm_maps(cfg)

    plan = []
    gidx_cols = [[] for _ in range(NCORES)]
    sr_cols = [[] for _ in range(NCORES)]
    # per-core dinv in (partition, window) layout, one per sub-graph
    dinv_w = np.zeros((NCORES, K, P, NT), np.float32)

    for k in range(K):
        r = np.asarray(rows[k]).astype(np.int64)
        c = np.asarray(cols[k]).astype(np.int64)
        src = np.concatenate([r, c + NU])
        dst = np.concatenate([c + NU, r])
        deg = np.bincount(src, minlength=N).astype(np.float32) + np.float32(1e-7)
        dinv = deg ** np.float32(-0.5)
        for m in range(NCORES):
            sel = core_of == m
            loc = loc_of[sel]
            dv = np.zeros(cfg.SHP, np.float32)
            dv[loc] = dinv[sel]
            dinv_w[m, k] = dv.reshape(NT, P).T

        e_core = core_of[src]
        e_tloc = loc_of[src]                  # accumulation target (local row)
        e_grow = g_of[dst]                    # gather row (padded global)
        e_b = np.searchsorted(cfg.BKS, e_grow, side="right") - 1
        e_w = e_tloc // P

        # per-core, per-(bucket, window) counts -> shared tile layout
        cell_cnt = np.zeros((NCORES, NB, NT), np.int64)
        flat = (e_core * NB + e_b) * NT + e_w
        np.add.at(cell_cnt.reshape(-1), flat, 1)
        nt_k = np.ceil(cell_cnt.max(axis=0) / P).astype(np.int64)  # [NB, NT]
        # every (bucket, octet) group needs >=1 tile so its PSUM bank is
        # zeroed and the acc copy/add happens
        for b in range(NB):
            for o in range(NO):
                if nt_k[b, o * OCT:(o + 1) * OCT].sum() == 0:
                    nt_k[b, o * OCT] = 1

        # BUCKET-MAJOR stream: bucket b's whole sweep only depends on
        # AllGather b, so the in-order gather queue never head-blocks on a
        # later bucket's collective.  One chunk list per bucket.
        chunks_k = [[] for _ in range(NB)]
        for b in range(NB):
            tot = int(nt_k[b].sum()) * P
            while tot > 0:
                L = min(CH, tot)
                chunks_k[b].append(L)
                tot -= L

        # per-tile schedule: PSUM accumulation group per (bucket, octet);
        # bucket 0 groups ACT-copy into acc, later buckets DVE-add.
        sched = []
        for b in range(NB):
            for o in range(NO):
                o_tot = int(nt_k[b, o * OCT:(o + 1) * OCT].sum())
                seen = 0
                for w in range(o * OCT, min((o + 1) * OCT, NT)):
                    for _ in range(int(nt_k[b, w])):
                        seen += 1
                        sched.append((o, w - o * OCT, seen == 1,
                                      seen == o_tot))
        plan.append({"nt": nt_k, "chunks": chunks_k, "sched": sched})

        order = np.lexsort((e_grow, e_w, e_b, e_core))
        s_core = e_core[order]
        s_tloc = e_tloc[order]
        s_grow = e_grow[order]
        s_b = e_b[order]
        s_w = e_w[order]
        start = np.searchsorted(s_core, np.arange(NCORES))
        stop = np.searchsorted(s_core, np.arange(NCORES) + 1)

        for m in range(NCORES):
            sl = slice(start[m], stop[m])
            mb, mw = s_b[sl], s_w[sl]
            mg, mt = s_grow[sl], s_tloc[sl]
            key = mb * NT + mw
            cs = np.searchsorted(key, np.arange(NB * NT))
            ce = np.searchsorted(key, np.arange(NB * NT) + 1)
            gi_parts, sr_parts = [], []
            for b in range(NB):
                for w in range(NT):
                    npad = int(nt_k[b, w]) * P
                    if npad == 0:
                        continue
                    i = b * NT + w
                    a, z = cs[i], ce[i]
                    n = z - a
                    gi = np.zeros(npad, np.int64)
                    sr = np.full(npad, -1.0, np.float32)  # pad: no target
                    gi[:n] = mg[a:z] - cfg.BKS[b]
                    sr[:n] = (mt[a:z] - w * P).astype(np.float32)
                    gi_parts.append(gi)
                    sr_parts.append(sr)
            gi_all = np.concatenate(gi_parts)
            sr_all = np.concatenate(sr_parts)
            off = 0
            for b in range(NB):
                for L in chunks_k[b]:
                    seg = gi_all[off:off + L].astype(np.int16)
                    seg = seg.reshape(L // 16, 16).T          # [16, L/16]
                    gidx_cols[m].append(np.tile(seg, (P // 16, 1)))
                    sr_cols[m].append(
                        sr_all[off:off + L].reshape(L // P, P).T
                        .astype(np.float16))
                    off += L
            assert off == len(gi_all)

    per_core = []
    for m in range(NCORES):
        per_core.append({
            "gidx": np.ascontiguousarray(np.concatenate(gidx_cols[m], axis=1)),
            "sr": np.ascontiguousarray(np.concatenate(sr_cols[m], axis=1)),
            "dinvw": np.ascontiguousarray(dinv_w[m]),
            "dinv2w": np.ascontiguousarray(dinv_w[m] ** 2),
        })
    return plan, per_core


def build(cfg, plan):
    nc = bacc.Bacc("TRN2", target_bir_lowering=False, debug=False,
                   num_devices=NCORES, num_swdge_queues=4,
                   dynamic_dma_scratch_size=131072)
    K, SHP, GN, NT, NO, CH = cfg.K, cfg.SHP, cfg.GN, cfg.NT, cfg.NO, cfg.CH
    TOTCOL = sum(L for k in range(K) for b in range(cfg.NB)
                 for L in plan[k]["chunks"][b])

    xT_in = nc.declare_dram_parameter("xT", [D, SHP], F32, isOutput=False)
    biascol = nc.declare_dram_parameter("biascol", [D, 1], F32, isOutput=False)
    fc1_WT = nc.declare_dram_parameter("fc1_WT", [D, D], F32, isOutput=False)
    fus1_WT = nc.declare_dram_parameter("fus1_WT", [D, D], F32, isOutput=False)
    b1col = nc.declare_dram_parameter("b1col", [D, 1], F32, isOutput=False)
    w2col = nc.declare_dram_parameter("w2col", [D, 1], F32, isOutput=False)
    gidx_d = nc.declare_dram_parameter("gidx", [P, TOTCOL // 16], I16,
                                       isOutput=False)
    sr_d = nc.declare_dram_parameter("sr", [P, TOTCOL // P], F16,
                                     isOutput=False)
    dinvw_d = nc.declare_dram_parameter("dinvw", [K, P, NT], F32,
                                        isOutput=False)
    dinv2w_d = nc.declare_dram_parameter("dinv2w", [K, P, NT], F32,
                                         isOutput=False)

    nodesT_o = nc.declare_dram_parameter("nodesT", [K, D, SHP], F32,
                                         isOutput=True)
    edges_o = nc.declare_dram_parameter("edges", [K, SHP, D], F32,
                                        isOutput=True)

    h_bounce = nc.dram_tensor("h_bounce", [SHP, TW], F16)
    e_bounce = nc.dram_tensor("e_bounce", [SHP, TW], F16)
    h_full = [nc.dram_tensor(f"h_full{b}", [cfg.BK[b], TW], F16,
                             addr_space="Shared") for b in range(cfg.NB)]
    e_full = [nc.dram_tensor(f"e_full{b}", [cfg.BK[b], TW], F16,
                             addr_space="Shared") for b in range(cfg.NB)]
    xT_d = nc.dram_tensor("xT_d", [D, SHP], F32)

    RG = [list(range(NCORES))]
    ccols = []
    o = 0
    while o < SHP:
        ccols.append((o, min(CCH, SHP - o)))
        o += min(CCH, SHP - o)

    with tile.TileContext(nc) as tc:
        with tc.tile_pool(name="persist", bufs=1) as pp:
            ident = pp.tile([P, P], F32)
            make_identity(nc, ident[:])
            ident16 = pp.tile([P, P], F16)
            nc.vector.tensor_copy(ident16[:], ident[:])
            iota_i = pp.tile([P, P], I32)
            nc.gpsimd.iota(iota_i[:], pattern=[[1, P]], base=0,
                           channel_multiplier=0)
            iota16 = pp.tile([P, P], F16)
            nc.vector.tensor_copy(iota16[:], iota_i[:])
            acc = pp.tile([P, NT * D], F32)
            wfc1 = pp.tile([D, D], F32)
            nc.sync.dma_start(wfc1[:], fc1_WT[:, :])
            wfus = pp.tile([D, D], F32)
            nc.sync.dma_start(wfus[:], fus1_WT[:, :])
            bcol = pp.tile([D, 1], F32)
            nc.sync.dma_start(bcol[:], biascol[:, :])
            b1c = pp.tile([D, 1], F32)
            nc.sync.dma_start(b1c[:], b1col[:, :])
            w2c = pp.tile([D, 1], F32)
            nc.sync.dma_start(w2c[:], w2col[:, :])
            ones1 = pp.tile([1, D], F32)
            nc.vector.memset(ones1[:], 1.0)

            nc.sync.dma_start(xT_d[:, :], xT_in[:, :])

            qctr = [0]

            def spmm(k, ph, table, col_off, on_octet=None):
                """One A @ table pass, bucket-major; result accumulates in
                acc (fp32): bucket 0 octet-groups ACT-copy into acc, later
                buckets DVE-add (DVE reads PSUM), so bucket b's gathers only
                ever wait on AllGather b."""
                sched = plan[k]["sched"]
                ti = 0
                with tc.tile_pool(name=f"sp{k}{ph}", bufs=3) as sp, \
                     tc.tile_pool(name=f"spS{k}{ph}", bufs=5) as spS, \
                     tc.tile_pool(name=f"spP{k}{ph}", bufs=4,
                                  space="PSUM") as spP:
                    PS = None
                    for b in range(cfg.NB):
                        nrow = cfg.BK[b]
                        for L in plan[k]["chunks"][b]:
                            nt_ch = L // P
                            gi = sp.tile([P, CH // 16], I16, tag="gi")
                            nc.sync.dma_start(
                                gi[:, 0:L // 16],
                                gidx_d[:, col_off // 16:
                                       (col_off + L) // 16])
                            sr = sp.tile([P, CH // P], F16, tag="sr")
                            nc.sync.dma_start(
                                sr[:, 0:nt_ch],
                                sr_d[:, col_off // P:(col_off + L) // P])
                            G = sp.tile([P, CH // P, TW], F16, tag="G")
                            nc.gpsimd.dma_gather(
                                out_ap=G[:, 0:nt_ch, :],
                                in_ap=table[b][0:nrow, :],
                                idxs_ap=gi[:, 0:L // 16],
                                num_idxs=L, num_idxs_reg=L, elem_size=TW,
                                single_packet=False,
                                queue_num=qctr[0] % 4)
                            qctr[0] += 1
                            S_tiles = []
                            for t0 in range(0, nt_ch, B_S):
                                bs = min(B_S, nt_ch - t0)
                                S = spS.tile([P, B_S, P], F16, tag="S")
                                nc.vector.tensor_tensor(
                                    out=S[:, 0:bs, :],
                                    in0=sr[:, t0:t0 + bs].unsqueeze(2)
                                        .to_broadcast([P, bs, P]),
                                    in1=iota16[:].unsqueeze(1)
                                        .to_broadcast([P, bs, P]),
                                    op=mybir.AluOpType.is_equal)
                                S_tiles.append(S)
                            for t in range(nt_ch):
                                o, wl, first, last = sched[ti]
                                ti += 1
                                if first:
                                    PS = spP.tile([P, OCT * D], F32,
                                                  tag="PS")
                                S = S_tiles[t // B_S]
                                nc.tensor.matmul(
                                    out=PS[:, wl * D:(wl + 1) * D],
                                    lhsT=S[:, t % B_S, :],
                                    rhs=G[:, t, 0:D],
                                    start=first, stop=last)
                                if last:
                                    wo = min((o + 1) * OCT, NT) - o * OCT
                                    asl = acc[:, o * OCT * D:
                                              (o * OCT + wo) * D]
                                    if b == 0:
                                        nc.scalar.activation(
                                            asl, PS[:, 0:wo * D], AF.Copy)
                                    else:
                                        nc.vector.tensor_tensor(
                                            out=asl, in0=asl,
                                            in1=PS[:, 0:wo * D],
                                            op=mybir.AluOpType.add)
                                    if (b == cfg.NB - 1
                                            and on_octet is not None):
                                        on_octet(o)
                            col_off += L
                    assert ti == len(sched)
                return col_off

            def h_chunk(sb, ps, xnew_s, co, cn, bounce, dv):
                """h rows = dinv * (relu(x) + bias), written fp16."""
                h_s = sb.tile([D, CCH], F32, tag="hs")
                nc.scalar.activation(h_s[:, 0:cn], xnew_s[:, 0:cn], AF.Relu)
                nc.vector.tensor_scalar_add(h_s[:, 0:cn], h_s[:, 0:cn],
                                            bcol[:, 0:1])
                for j in range(0, cn, P):
                    pj = min(P, cn - j)
                    w = (co + j) // P
                    pst = ps.tile([P, D], F32, tag="hT")
                    nc.tensor.transpose(pst[0:pj, :], h_s[:, j:j + pj],
                                        ident[0:D, 0:D])
                    hr = sb.tile([P, D], F16, tag="hr")
                    nc.scalar.activation(hr[0:pj, :], pst[0:pj, :], AF.Copy,
                                         scale=dv[0:pj, w:w + 1])
                    nc.sync.dma_start(bounce[co + j:co + j + pj, 0:D],
                                      hr[0:pj, :])

            dv_tiles = []
            for k in range(K):
                dv = pp.tile([P, NT], F32, tag=f"dv{k}")
                nc.sync.dma_start(dv[:], dinvw_d[k, :, :])
                dv2 = pp.tile([P, NT], F32, tag=f"dv2{k}")
                nc.sync.dma_start(dv2[:], dinv2w_d[k, :, :])
                dv_tiles.append((dv, dv2))

            def ag_q(bounce, full, q):
                rs = int(cfg.QRS[q])
                nc.gpsimd.collective_compute(
                    "AllGather", mybir.AluOpType.bypass, replica_groups=RG,
                    ins=[bounce[rs:rs + cfg.QR[q], :].opt()],
                    outs=[full[q].ap().opt()])

            # quarter q's windows end inside this octet
            q_done_at = {}
            for q in range(cfg.NB):
                wend = sum(cfg.QW[:q + 1])
                q_done_at.setdefault((wend - 1) // OCT, []).append(q)

            # initial h from input x; AG each quarter as soon as written
            # (Pool queue is empty here, so no gather head-blocking)
            with tc.tile_pool(name="h0", bufs=3) as hp, \
                 tc.tile_pool(name="h0p", bufs=2, space="PSUM") as hpp:
                hq = 0
                for (co, cn) in ccols:
                    xc_s = hp.tile([D, CCH], F32, tag="xc")
                    nc.sync.dma_start(xc_s[:, 0:cn], xT_d[:, co:co + cn])
                    h_chunk(hp, hpp, xc_s, co, cn, h_bounce, dv_tiles[0][0])
                    while hq < cfg.NB and co + cn >= int(cfg.QRS[hq + 1]):
                        ag_q(h_bounce, h_full, hq)
                        hq += 1
                assert hq == cfg.NB

            col_off = 0
            for k in range(K):
                dv, dv2 = dv_tiles[k]
                with tc.tile_pool(name=f"ec{k}", bufs=1) as ec:
                    # compute each e-table quarter (dinv^2 * acc, fp16) and
                    # its bounce DMA as soon as spmm0's octets cover it; its
                    # AllGather is issued one octet LATER, so the collective's
                    # input wait is already satisfied (no Pool head-block)
                    # and the AG overlaps spmm0's remaining gathers instead
                    # of running serially after spmm0.
                    pend_ag = []

                    def e_hook(o, dv2=dv2, ec=ec):
                        if pend_ag and o > pend_ag[0][1]:
                            ag_q(e_bounce, e_full, pend_ag.pop(0)[0])
                        for q in q_done_at.get(o, []):
                            w0 = sum(cfg.QW[:q])
                            qw = cfg.QW[q]
                            e16 = ec.tile([P, 32 * D], F16, tag="e16")
                            nc.vector.tensor_tensor(
                                out=e16[:, 0:qw * D].rearrange(
                                    "p (w d) -> p w d", d=D),
                                in0=acc[:, w0 * D:(w0 + qw) * D].rearrange(
                                    "p (w d) -> p w d", d=D),
                                in1=dv2[:, w0:w0 + qw].unsqueeze(2)
                                    .to_broadcast([P, qw, D]),
                                op=mybir.AluOpType.mult)
                            rs = int(cfg.QRS[q])
                            nc.sync.dma_start(
                                e_bounce[rs:rs + cfg.QR[q], 0:D].rearrange(
                                    "(w p) d -> p w d", p=P),
                                e16[:, 0:qw * D].rearrange(
                                    "p (w d) -> p w d", d=D))
                            pend_ag.append((q, o))

                    spmm(k, 0, h_full, col_off, on_octet=e_hook)
                    for q, _ in pend_ag:
                        ag_q(e_bounce, e_full, q)
                # fp32 edge output = dinv * acc (off the critical path)
                with tc.tile_pool(name=f"eo{k}", bufs=1) as eo:
                    e32 = eo.tile([P, NT * D], F32, tag="e32")
                    nc.vector.tensor_tensor(
                        out=e32[:].rearrange("p (w d) -> p w d", d=D),
                        in0=acc[:].rearrange("p (w d) -> p w d", d=D),
                        in1=dv[:].unsqueeze(2).to_broadcast([P, NT, D]),
                        op=mybir.AluOpType.mult)
                    nc.sync.dma_start(
                        edges_o[k, :, :].rearrange("(w p) d -> p w d", p=P),
                        e32[:].rearrange("p (w d) -> p w d", d=D))
                col_off = spmm(k, 1, e_full, col_off)

                with tc.tile_pool(name=f"po{k}", bufs=3) as po, \
                     tc.tile_pool(name=f"poP{k}", bufs=2, space="PSUM") as poP:
                    # node = dinv * acc, folded in ahead of the softmax
                    a3 = acc[:].rearrange("p (w d) -> p w d", d=D)
                    nc.vector.tensor_tensor(
                        out=a3, in0=a3,
                        in1=dv[:].unsqueeze(2).to_broadcast([P, NT, D]),
                        op=mybir.AluOpType.mult)
                    nbv = 14
                    for w0 in range(0, NT, nbv):
                        bw = min(nbv, NT - w0)
                        sl = acc[:, w0 * D:(w0 + bw) * D]
                        sl3 = sl.rearrange("p (b d) -> p b d", d=D)
                        nc.scalar.activation(sl, sl, AF.Exp)
                        ssum = po.tile([P, nbv], F32, tag="ssum")
                        nc.vector.reduce_sum(ssum[:, 0:bw], sl3,
                                             axis=mybir.AxisListType.X)
                        nc.vector.reciprocal(ssum[:, 0:bw], ssum[:, 0:bw])
                        nc.vector.tensor_tensor(
                            out=sl3, in0=sl3,
                            in1=ssum[:, 0:bw].unsqueeze(2).to_broadcast(
                                [P, bw, D]),
                            op=mybir.AluOpType.mult)
                    hq = 0
                    for (co, cn) in ccols:
                        psT = poP.tile([D, CCH], F32, tag="T")
                        for j in range(0, cn, P):
                            pj = min(P, cn - j)
                            w = (co + j) // P
                            nc.tensor.transpose(
                                psT[:, j:j + pj],
                                acc[:, w * D:(w + 1) * D][0:pj, :],
                                ident[0:pj, 0:pj])
                        smT = po.tile([D, CCH], F32, tag="smT")
                        nc.scalar.activation(smT[:, 0:cn], psT[:, 0:cn],
                                             AF.Copy)
                        psN = poP.tile([D, CCH], F32, tag="N")
                        nc.tensor.matmul(psN[:, 0:cn], lhsT=wfc1[:, :],
                                         rhs=smT[:, 0:cn], start=True,
                                         stop=True)
                        nodeT = po.tile([D, CCH], F32, tag="nodeT")
                        nc.scalar.activation(nodeT[:, 0:cn], psN[:, 0:cn],
                                             AF.Copy)
                        xc_s = po.tile([D, CCH], F32, tag="xc")
                        nc.sync.dma_start(xc_s[:, 0:cn], xT_d[:, co:co + cn])
                        psG = poP.tile([D, CCH], F32, tag="T")
                        nc.tensor.matmul(psG[:, 0:cn], lhsT=wfus[:, :],
                                         rhs=xc_s[:, 0:cn], start=True,
                                         stop=True)
                        t1x = po.tile([D, CCH], F32, tag="t1x")
                        nc.scalar.activation(t1x[:, 0:cn], psG[:, 0:cn],
                                             AF.Tanh, bias=b1c[:, 0:1])
                        psA0 = poP.tile([1, CCH], F32, tag="A")
                        nc.tensor.matmul(psA0[:, 0:cn], lhsT=w2c[:, :],
                                         rhs=t1x[:, 0:cn], start=True,
                                         stop=True)
                        psG2 = poP.tile([D, CCH], F32, tag="N")
                        nc.tensor.matmul(psG2[:, 0:cn], lhsT=wfus[:, :],
                                         rhs=nodeT[:, 0:cn], start=True,
                                         stop=True)
                        t1n = po.tile([D, CCH], F32, tag="t1n")
                        nc.scalar.activation(t1n[:, 0:cn], psG2[:, 0:cn],
                                             AF.Tanh, bias=b1c[:, 0:1])
                        psA1 = poP.tile([1, CCH], F32, tag="A")
                        nc.tensor.matmul(psA1[:, 0:cn], lhsT=w2c[:, :],
                                         rhs=t1n[:, 0:cn], start=True,
                                         stop=True)
                        a1s = po.tile([1, CCH], F32, tag="a1s")
                        nc.scalar.activation(a1s[:, 0:cn], psA1[:, 0:cn],
                                             AF.Copy)
                        s0 = po.tile([1, CCH], F32, tag="s0")
                        nc.vector.tensor_tensor(out=s0[:, 0:cn],
                                                in0=psA0[:, 0:cn],
                                                in1=a1s[:, 0:cn],
                                                op=mybir.AluOpType.subtract)
                        nc.scalar.activation(s0[:, 0:cn], s0[:, 0:cn],
                                             AF.Sigmoid)
                        s0b = poP.tile([D, CCH], F32, tag="A")
                        nc.tensor.matmul(s0b[:, 0:cn], lhsT=ones1[:, :],
                                         rhs=s0[:, 0:cn], start=True,
                                         stop=True)
                        diff = po.tile([D, CCH], F32, tag="diff")
                        nc.vector.tensor_tensor(out=diff[:, 0:cn],
                                                in0=xc_s[:, 0:cn],
                                                in1=nodeT[:, 0:cn],
                                                op=mybir.AluOpType.subtract)
                        nc.vector.tensor_tensor(
                            out=diff[:, 0:cn], in0=diff[:, 0:cn],
                            in1=s0b[:, 0:cn],
                            op=mybir.AluOpType.mult)
                        xnew = po.tile([D, CCH], F32, tag="xnew")
                        nc.vector.tensor_tensor(out=xnew[:, 0:cn],
                                                in0=nodeT[:, 0:cn],
                                                in1=diff[:, 0:cn],
                                                op=mybir.AluOpType.add)
                        nc.sync.dma_start(xT_d[:, co:co + cn], xnew[:, 0:cn])
                        nc.sync.dma_start(nodesT_o[k, :, co:co + cn],
                                          xnew[:, 0:cn])
                        if k < K - 1:
                            h_chunk(po, poP, xnew, co, cn, h_bounce,
                                    dv_tiles[k + 1][0])
                            while (hq < cfg.NB
                                   and co + cn >= int(cfg.QRS[hq + 1])):
                                ag_q(h_bounce, h_full, hq)
                                hq += 1
    nc.compile()
    return nc


_CACHE = {}


def _plan_key(plan):
    return tuple(
        (tuple(map(tuple, pk["nt"])),
         tuple(tuple(cb) for cb in pk["chunks"]))
        for pk in plan)


def _get_nc(cfg, plan):
    key = (cfg.NU, cfg.NI, cfg.K, cfg.E, cfg.CH, _plan_key(plan))
    if key not in _CACHE:
        _CACHE[key] = build(cfg, plan)
    return _CACHE[key]


def prepare(cfg, x, hgc1_bias, fc1_W, fus_l1_W, fus_l1_b, fus_l2_W, fus_l2_b,
            rows, cols):
    """Host prep: build (cached) module + per-core input maps."""
    x = np.asarray(x, np.float32)
    plan, per_core = prep(cfg, rows, cols)
    nc = _get_nc(cfg, plan)

    core_of, loc_of, _ = _perm_maps(cfg)
    in_maps = []
    for m in range(NCORES):
        xm = np.zeros((cfg.SHP, D), np.float32)
        sel = core_of == m
        xm[loc_of[sel]] = x[sel]
        in_maps.append({
            "xT": np.ascontiguousarray(xm.T),
            "biascol": np.asarray(hgc1_bias, np.float32).reshape(D, 1),
            "fc1_WT": np.ascontiguousarray(np.asarray(fc1_W, np.float32).T),
            "fus1_WT": np.ascontiguousarray(np.asarray(fus_l1_W, np.float32).T),
            "b1col": np.asarray(fus_l1_b, np.float32).reshape(D, 1),
            "w2col": np.ascontiguousarray(
                np.asarray(fus_l2_W, np.float32).reshape(1, D).T),
            "gidx": per_core[m]["gidx"],
            "sr": per_core[m]["sr"],
            "dinvw": per_core[m]["dinvw"],
            "dinv2w": per_core[m]["dinv2w"],
        })
    return nc, in_maps


def unshard(cfg, results):
    """Per-core outputs -> full (nodes, edges)."""
    core_of, loc_of, _ = _perm_maps(cfg)
    N = cfg.NU + cfg.NI
    nodes = np.zeros((cfg.K, N, D), np.float32)
    edges = np.zeros((cfg.K, N, D), np.float32)
    for m in range(NCORES):
        sel = core_of == m
        nodesT = np.asarray(results[m]["nodesT"]).reshape(cfg.K, D, cfg.SHP)
        edg = np.asarray(results[m]["edges"]).reshape(cfg.K, cfg.SHP, D)
        nodes[:, sel, :] = nodesT[:, :, loc_of[sel]].transpose(0, 2, 1)
        edges[:, sel, :] = edg[:, loc_of[sel], :]
    return nodes, edges


def run(cfg, x, hgc1_bias, fc1_W, fus_l1_W, fus_l1_b, fus_l2_W, fus_l2_b,
        rows, cols, sim=False):
    nc, in_maps = prepare(cfg, x, hgc1_bias, fc1_W, fus_l1_W, fus_l1_b,
                          fus_l2_W, fus_l2_b, rows, cols)

    if sim:
        from concourse import bass_interp
        simu = bass_interp.MultiCoreSim(nc, NCORES, require_finite=False,
                                        require_nnan=False)
        for m in range(NCORES):
            for kk, a in in_maps[m].items():
                simu.cores[m].tensor(kk)[:] = a
        simu.simulate()
        results = [{"nodesT": simu.cores[m].mem_tensor("nodesT"),
                    "edges": simu.cores[m].mem_tensor("edges")}
                   for m in range(NCORES)]
        exec_ns = None
    else:
        from concourse.bass_utils import run_bass_kernel_spmd
        res = run_bass_kernel_spmd(nc, in_maps, core_ids=list(range(NCORES)))
        results = res.results
        exec_ns = res.exec_time_ns

    return unshard(cfg, results), exec_ns


def kernel(x, hgc1_bias, fc1_W, fus_l1_W, fus_l1_b, fus_l2_W, fus_l2_b,
           rows, cols):
    cfg = Cfg(NU=50000, NI=50000, K=4, E=1000000, CH=6144)
    (nodes, edges), _ = run(cfg, x, hgc1_bias, fc1_W, fus_l1_W, fus_l1_b,
                            fus_l2_W, fus_l2_b, rows, cols)
    return nodes, edges


# revision 34
# speedup vs baseline: 1.0205x; 1.0205x over previous
"""Trainium2 Bass kernel for nn_HGNN_ATT (HGNN message passing, K sub-graphs).

Sharding: nodes re-permuted so every shard holds an equal mix of users and
items (shard m = users [6250m..) ++ items [6250m..)), padded to 12544 rows.
Each directed edge is owned by the core owning its accumulation target.

spmm datapath (fp16): AllGather h (fp16 rows padded to 128 cols so each
gather descriptor is 256B) -> dma_gather chunks -> DVE scale by val ->
one-hot S per 128-edge tile (is_equal vs iota, fp16) -> PE fp16 matmul into
a 2KB PSUM bank holding one window-octet (8 windows x 64 cols).  Edges are
sorted (octet, bucket, window) so an octet's tiles across all buckets form
ONE PSUM accumulation group (start=True zeroes the whole 2KB zero region on
the octet's first tile; stop=True only on its last) -> one ACT copy per
octet writes PSUM to the SBUF acc.  Softmax/fc1/fusion-gate run in
feature-major fp32 layout as before.
"""

import numpy as np

import concourse.bass as bass
import concourse.mybir as mybir
import concourse.tile as tile
from concourse import bacc
from concourse.masks import make_identity

F32 = mybir.dt.float32
F16 = mybir.dt.float16
I16 = mybir.dt.int16
I32 = mybir.dt.int32
AF = mybir.ActivationFunctionType

NCORES = 8
D = 64
P = 128
TW = 128                # padded table row width (TW*2B = 256B descriptors)
BUCKET = 32768          # int16 gather index range
B_S = 16                # tiles per S-build DVE instruction
CCH = 512               # post-phase column chunk (PSUM free-dim limit)


OCT = 8                 # windows per PSUM zero region (2KB bank / 256B)


class Cfg:
    def __init__(self, NU, NI, K, E, CH):
        assert NU % NCORES == 0 and NI % NCORES == 0
        self.NU, self.NI, self.K, self.E, self.CH = NU, NI, K, E, CH
        self.UPC = NU // NCORES
        self.IPC = NI // NCORES
        self.SH = self.UPC + self.IPC            # real rows per shard
        self.SHP = ((self.SH + P - 1) // P) * P  # padded rows per shard
        self.NT = self.SHP // P                  # target windows per shard
        self.GN = NCORES * self.SHP              # padded global table rows
        self.NO = (self.NT + OCT - 1) // OCT     # window octets
        # buckets = local-window quarters, so each bucket's table slice is
        # filled by its own (small) AllGather and gathers overlap collectives
        self.NB = min(4, self.NT)
        wq = [self.NT // self.NB + (1 if i < self.NT % self.NB else 0)
              for i in range(self.NB)]
        self.QW = wq                                  # windows per quarter
        self.QR = [w * P for w in wq]                 # local rows per quarter
        self.QRS = np.concatenate([[0], np.cumsum(self.QR)]).astype(np.int64)
        self.BK = [NCORES * r for r in self.QR]       # bucket table rows
        self.BKS = np.concatenate([[0], np.cumsum(self.BK)]).astype(np.int64)
        assert max(self.BK) <= 32768                  # int16 gather indices


def _perm_maps(cfg):
    """original node id -> (core, local_row, padded_global_row).

    Global gather rows are quarter-major: bucket b holds every core's
    quarter-b rows (core-major inside the bucket)."""
    orig = np.arange(cfg.NU + cfg.NI)
    is_item = orig >= cfg.NU
    core = np.where(is_item, (orig - cfg.NU) // cfg.IPC, orig // cfg.UPC)
    loc = np.where(is_item, cfg.UPC + (orig - cfg.NU) % cfg.IPC, orig % cfg.UPC)
    q = np.searchsorted(cfg.QRS, loc, side="right") - 1
    g = cfg.BKS[q] + core * np.asarray(cfg.QR)[q] + (loc - cfg.QRS[q])
    return core.astype(np.int64), loc.astype(np.int64), g.astype(np.int64)


def prep(cfg, rows, cols):
    """Host-side graph preprocessing.

    Returns (plan, per-core arrays).  plan[k] holds per-(octet, bucket)
    chunk lists and the per-tile (w_local, start, stop) schedule shared by
    both spmm phases of sub-graph k.
    """
    NU, NI, K, CH = cfg.NU, cfg.NI, cfg.K, cfg.CH
    N = NU + NI
    NB, NT, NO = cfg.NB, cfg.NT, cfg.NO
    core_of, loc_of, g_of = _perm_maps(cfg)

    plan = []
    gidx_cols = [[] for _ in range(NCORES)]
    sr_cols = [[] for _ in range(NCORES)]
    # per-core dinv in (partition, window) layout, one per sub-graph
    dinv_w = np.zeros((NCORES, K, P, NT), np.float32)

    for k in range(K):
        r = np.asarray(rows[k]).astype(np.int64)
        c = np.asarray(cols[k]).astype(np.int64)
        src = np.concatenate([r, c + NU])
        dst = np.concatenate([c + NU, r])
        deg = np.bincount(src, minlength=N).astype(np.float32) + np.float32(1e-7)
        dinv = deg ** np.float32(-0.5)
        for m in range(NCORES):
            sel = core_of == m
            loc = loc_of[sel]
            dv = np.zeros(cfg.SHP, np.float32)
            dv[loc] = dinv[sel]
            dinv_w[m, k] = dv.reshape(NT, P).T

        e_core = core_of[src]
        e_tloc = loc_of[src]                  # accumulation target (local row)
        e_grow = g_of[dst]                    # gather row (padded global)
        e_b = np.searchsorted(cfg.BKS, e_grow, side="right") - 1
        e_w = e_tloc // P

        # per-core, per-(bucket, window) counts -> shared tile layout
        cell_cnt = np.zeros((NCORES, NB, NT), np.int64)
        flat = (e_core * NB + e_b) * NT + e_w
        np.add.at(cell_cnt.reshape(-1), flat, 1)
        nt_k = np.ceil(cell_cnt.max(axis=0) / P).astype(np.int64)  # [NB, NT]
        # every octet needs >=1 tile so its PSUM bank is zeroed + copied
        for o in range(NO):
            if nt_k[:, o * OCT:(o + 1) * OCT].sum() == 0:
                nt_k[0, o * OCT] = 1

        # chunk lists per (octet, bucket)
        chunks_k = [[[] for _ in range(NB)] for _ in range(NO)]
        for o in range(NO):
            for b in range(NB):
                tot = int(nt_k[b, o * OCT:(o + 1) * OCT].sum()) * P
                while tot > 0:
                    L = min(CH, tot)
                    chunks_k[o][b].append(L)
                    tot -= L

        # per-tile schedule in stream order (same for every core).  An
        # octet's tiles (across all buckets) form one PSUM accumulation
        # group: start=True (zeroes the 2KB region) only on the octet's
        # first tile, stop=True only on its last.
        sched = []
        for o in range(NO):
            o_tot = int(nt_k[:, o * OCT:(o + 1) * OCT].sum())
            seen = 0
            for b in range(NB):
                for w in range(o * OCT, min((o + 1) * OCT, NT)):
                    for _ in range(int(nt_k[b, w])):
                        seen += 1
                        sched.append((w - o * OCT, seen == 1, seen == o_tot))
        plan.append({"nt": nt_k, "chunks": chunks_k, "sched": sched})

        e_o = e_w // OCT
        order = np.lexsort((e_grow, e_w, e_b, e_o, e_core))
        s_core = e_core[order]
        s_tloc = e_tloc[order]
        s_grow = e_grow[order]
        s_b = e_b[order]
        s_w = e_w[order]
        start = np.searchsorted(s_core, np.arange(NCORES))
        stop = np.searchsorted(s_core, np.arange(NCORES) + 1)

        for m in range(NCORES):
            sl = slice(start[m], stop[m])
            mb, mw = s_b[sl], s_w[sl]
            mg, mt = s_grow[sl], s_tloc[sl]
            key = ((mw // OCT) * NB + mb) * NT + mw
            cs = np.searchsorted(key, np.arange(NO * NB * NT))
            ce = np.searchsorted(key, np.arange(NO * NB * NT) + 1)
            gi_parts, sr_parts = [], []
            for o in range(NO):
                for b in range(NB):
                    for w in range(o * OCT, min((o + 1) * OCT, NT)):
                        npad = int(nt_k[b, w]) * P
                        if npad == 0:
                            continue
                        i = (o * NB + b) * NT + w
                        a, z = cs[i], ce[i]
                        n = z - a
                        gi = np.zeros(npad, np.int64)
                        sr = np.full(npad, -1.0, np.float32)  # pad: no target
                        gi[:n] = mg[a:z] - cfg.BKS[b]
                        sr[:n] = (mt[a:z] - w * P).astype(np.float32)
                        gi_parts.append(gi)
                        sr_parts.append(sr)
            gi_all = np.concatenate(gi_parts)
            sr_all = np.concatenate(sr_parts)
            off = 0
            for o in range(NO):
                for b in range(NB):
                    for L in chunks_k[o][b]:
                        seg = gi_all[off:off + L].astype(np.int16)
                        seg = seg.reshape(L // 16, 16).T          # [16, L/16]
                        gidx_cols[m].append(np.tile(seg, (P // 16, 1)))
                        sr_cols[m].append(
                            sr_all[off:off + L].reshape(L // P, P).T
                            .astype(np.float16))
                        off += L
            assert off == len(gi_all)

    per_core = []
    for m in range(NCORES):
        per_core.append({
            "gidx": np.ascontiguousarray(np.concatenate(gidx_cols[m], axis=1)),
            "sr": np.ascontiguousarray(np.concatenate(sr_cols[m], axis=1)),
            "dinvw": np.ascontiguousarray(dinv_w[m]),
            "dinv2w": np.ascontiguousarray(dinv_w[m] ** 2),
        })
    return plan, per_core


def build(cfg, plan):
    nc = bacc.Bacc("TRN2", target_bir_lowering=False, debug=False,
                   num_devices=NCORES, num_swdge_queues=4,
                   dynamic_dma_scratch_size=131072)
    K, SHP, GN, NT, NO, CH = cfg.K, cfg.SHP, cfg.GN, cfg.NT, cfg.NO, cfg.CH
    TOTCOL = sum(L for k in range(K) for o in range(NO)
                 for b in range(cfg.NB) for L in plan[k]["chunks"][o][b])

    xT_in = nc.declare_dram_parameter("xT", [D, SHP], F32, isOutput=False)
    biascol = nc.declare_dram_parameter("biascol", [D, 1], F32, isOutput=False)
    fc1_WT = nc.declare_dram_parameter("fc1_WT", [D, D], F32, isOutput=False)
    fus1_WT = nc.declare_dram_parameter("fus1_WT", [D, D], F32, isOutput=False)
    b1col = nc.declare_dram_parameter("b1col", [D, 1], F32, isOutput=False)
    w2col = nc.declare_dram_parameter("w2col", [D, 1], F32, isOutput=False)
    gidx_d = nc.declare_dram_parameter("gidx", [P, TOTCOL // 16], I16,
                                       isOutput=False)
    sr_d = nc.declare_dram_parameter("sr", [P, TOTCOL // P], F16,
                                     isOutput=False)
    dinvw_d = nc.declare_dram_parameter("dinvw", [K, P, NT], F32,
                                        isOutput=False)
    dinv2w_d = nc.declare_dram_parameter("dinv2w", [K, P, NT], F32,
                                         isOutput=False)

    nodesT_o = nc.declare_dram_parameter("nodesT", [K, D, SHP], F32,
                                         isOutput=True)
    edges_o = nc.declare_dram_parameter("edges", [K, SHP, D], F32,
                                        isOutput=True)

    h_bounce = nc.dram_tensor("h_bounce", [SHP, TW], F16)
    e_bounce = nc.dram_tensor("e_bounce", [SHP, TW], F16)
    h_full = [nc.dram_tensor(f"h_full{b}", [cfg.BK[b], TW], F16,
                             addr_space="Shared") for b in range(cfg.NB)]
    e_full = [nc.dram_tensor(f"e_full{b}", [cfg.BK[b], TW], F16,
                             addr_space="Shared") for b in range(cfg.NB)]
    xT_d = nc.dram_tensor("xT_d", [D, SHP], F32)

    RG = [list(range(NCORES))]
    ccols = []
    o = 0
    while o < SHP:
        ccols.append((o, min(CCH, SHP - o)))
        o += min(CCH, SHP - o)

    with tile.TileContext(nc) as tc:
        with tc.tile_pool(name="persist", bufs=1) as pp:
            ident = pp.tile([P, P], F32)
            make_identity(nc, ident[:])
            ident16 = pp.tile([P, P], F16)
            nc.vector.tensor_copy(ident16[:], ident[:])
            iota_i = pp.tile([P, P], I32)
            nc.gpsimd.iota(iota_i[:], pattern=[[1, P]], base=0,
                           channel_multiplier=0)
            iota16 = pp.tile([P, P], F16)
            nc.vector.tensor_copy(iota16[:], iota_i[:])
            acc = pp.tile([P, NT * D], F32)
            wfc1 = pp.tile([D, D], F32)
            nc.sync.dma_start(wfc1[:], fc1_WT[:, :])
            wfus = pp.tile([D, D], F32)
            nc.sync.dma_start(wfus[:], fus1_WT[:, :])
            bcol = pp.tile([D, 1], F32)
            nc.sync.dma_start(bcol[:], biascol[:, :])
            b1c = pp.tile([D, 1], F32)
            nc.sync.dma_start(b1c[:], b1col[:, :])
            w2c = pp.tile([D, 1], F32)
            nc.sync.dma_start(w2c[:], w2col[:, :])
            ones1 = pp.tile([1, D], F32)
            nc.vector.memset(ones1[:], 1.0)

            nc.sync.dma_start(xT_d[:, :], xT_in[:, :])

            qctr = [0]

            def spmm(k, ph, table, col_off, on_octet=None):
                """One A @ table pass; result lands in acc (fp32)."""
                sched = plan[k]["sched"]
                ti = 0
                with tc.tile_pool(name=f"sp{k}{ph}", bufs=3) as sp, \
                     tc.tile_pool(name=f"spS{k}{ph}", bufs=5) as spS, \
                     tc.tile_pool(name=f"spP{k}{ph}", bufs=4,
                                  space="PSUM") as spP:
                    for o in range(NO):
                        PS = spP.tile([P, OCT * D], F32, tag="PS")
                        for b in range(cfg.NB):
                            nrow = cfg.BK[b]
                            for L in plan[k]["chunks"][o][b]:
                                nt_ch = L // P
                                gi = sp.tile([P, CH // 16], I16, tag="gi")
                                nc.sync.dma_start(
                                    gi[:, 0:L // 16],
                                    gidx_d[:, col_off // 16:
                                           (col_off + L) // 16])
                                sr = sp.tile([P, CH // P], F16, tag="sr")
                                nc.sync.dma_start(
                                    sr[:, 0:nt_ch],
                                    sr_d[:, col_off // P:(col_off + L) // P])
                                G = sp.tile([P, CH // P, TW], F16, tag="G")
                                nc.gpsimd.dma_gather(
                                    out_ap=G[:, 0:nt_ch, :],
                                    in_ap=table[b][0:nrow, :],
                                    idxs_ap=gi[:, 0:L // 16],
                                    num_idxs=L, num_idxs_reg=L, elem_size=TW,
                                    single_packet=False,
                                    queue_num=qctr[0] % 4)
                                qctr[0] += 1
                                S_tiles = []
                                for t0 in range(0, nt_ch, B_S):
                                    bs = min(B_S, nt_ch - t0)
                                    S = spS.tile([P, B_S, P], F16, tag="S")
                                    nc.vector.tensor_tensor(
                                        out=S[:, 0:bs, :],
                                        in0=sr[:, t0:t0 + bs].unsqueeze(2)
                                            .to_broadcast([P, bs, P]),
                                        in1=iota16[:].unsqueeze(1)
                                            .to_broadcast([P, bs, P]),
                                        op=mybir.AluOpType.is_equal)
                                    S_tiles.append(S)
                                for t in range(nt_ch):
                                    wl, first, last = sched[ti]
                                    ti += 1
                                    S = S_tiles[t // B_S]
                                    nc.tensor.matmul(
                                        out=PS[:, wl * D:(wl + 1) * D],
                                        lhsT=S[:, t % B_S, :],
                                        rhs=G[:, t, 0:D],
                                        start=first, stop=last)
                                col_off += L
                        wo = min((o + 1) * OCT, NT) - o * OCT
                        nc.scalar.activation(
                            acc[:, o * OCT * D:(o * OCT + wo) * D],
                            PS[:, 0:wo * D], AF.Copy)
                        if on_octet is not None:
                            on_octet(o)
                    assert ti == len(sched)
                return col_off

            def h_chunk(sb, ps, xnew_s, co, cn, bounce, dv):
                """h rows = dinv * (relu(x) + bias), written fp16."""
                h_s = sb.tile([D, CCH], F32, tag="hs")
                nc.scalar.activation(h_s[:, 0:cn], xnew_s[:, 0:cn], AF.Relu)
                nc.vector.tensor_scalar_add(h_s[:, 0:cn], h_s[:, 0:cn],
                                            bcol[:, 0:1])
                for j in range(0, cn, P):
                    pj = min(P, cn - j)
                    w = (co + j) // P
                    pst = ps.tile([P, D], F32, tag="hT")
                    nc.tensor.transpose(pst[0:pj, :], h_s[:, j:j + pj],
                                        ident[0:D, 0:D])
                    hr = sb.tile([P, D], F16, tag="hr")
                    nc.scalar.activation(hr[0:pj, :], pst[0:pj, :], AF.Copy,
                                         scale=dv[0:pj, w:w + 1])
                    nc.sync.dma_start(bounce[co + j:co + j + pj, 0:D],
                                      hr[0:pj, :])

            dv_tiles = []
            for k in range(K):
                dv = pp.tile([P, NT], F32, tag=f"dv{k}")
                nc.sync.dma_start(dv[:], dinvw_d[k, :, :])
                dv2 = pp.tile([P, NT], F32, tag=f"dv2{k}")
                nc.sync.dma_start(dv2[:], dinv2w_d[k, :, :])
                dv_tiles.append((dv, dv2))

            def ag_q(bounce, full, q):
                rs = int(cfg.QRS[q])
                nc.gpsimd.collective_compute(
                    "AllGather", mybir.AluOpType.bypass, replica_groups=RG,
                    ins=[bounce[rs:rs + cfg.QR[q], :].opt()],
                    outs=[full[q].ap().opt()])

            # quarter q's windows end inside this octet
            q_done_at = {}
            for q in range(cfg.NB):
                wend = sum(cfg.QW[:q + 1])
                q_done_at.setdefault((wend - 1) // OCT, []).append(q)

            # initial h from input x; AG each quarter as soon as written
            # (Pool queue is empty here, so no gather head-blocking)
            with tc.tile_pool(name="h0", bufs=3) as hp, \
                 tc.tile_pool(name="h0p", bufs=2, space="PSUM") as hpp:
                hq = 0
                for (co, cn) in ccols:
                    xc_s = hp.tile([D, CCH], F32, tag="xc")
                    nc.sync.dma_start(xc_s[:, 0:cn], xT_d[:, co:co + cn])
                    h_chunk(hp, hpp, xc_s, co, cn, h_bounce, dv_tiles[0][0])
                    while hq < cfg.NB and co + cn >= int(cfg.QRS[hq + 1]):
                        ag_q(h_bounce, h_full, hq)
                        hq += 1
                assert hq == cfg.NB

            col_off = 0
            for k in range(K):
                dv, dv2 = dv_tiles[k]
                with tc.tile_pool(name=f"ec{k}", bufs=1) as ec:
                    # compute each e-table quarter (dinv^2 * acc, fp16) and
                    # its bounce DMA as soon as spmm0's octets cover it; the
                    # AllGathers are issued only after spmm0 so they never
                    # head-block the remaining gathers on the Pool queue,
                    # yet their inputs have already landed by then.
                    def e_hook(o, dv2=dv2, ec=ec):
                        for q in q_done_at.get(o, []):
                            w0 = sum(cfg.QW[:q])
                            qw = cfg.QW[q]
                            e16 = ec.tile([P, 32 * D], F16, tag="e16")
                            nc.vector.tensor_tensor(
                                out=e16[:, 0:qw * D].rearrange(
                                    "p (w d) -> p w d", d=D),
                                in0=acc[:, w0 * D:(w0 + qw) * D].rearrange(
                                    "p (w d) -> p w d", d=D),
                                in1=dv2[:, w0:w0 + qw].unsqueeze(2)
                                    .to_broadcast([P, qw, D]),
                                op=mybir.AluOpType.mult)
                            rs = int(cfg.QRS[q])
                            nc.sync.dma_start(
                                e_bounce[rs:rs + cfg.QR[q], 0:D].rearrange(
                                    "(w p) d -> p w d", p=P),
                                e16[:, 0:qw * D].rearrange(
                                    "p (w d) -> p w d", d=D))

                    spmm(k, 0, h_full, col_off, on_octet=e_hook)
                    for q in range(cfg.NB):
                        ag_q(e_bounce, e_full, q)
                # fp32 edge output = dinv * acc (off the critical path)
                with tc.tile_pool(name=f"eo{k}", bufs=1) as eo:
                    e32 = eo.tile([P, NT * D], F32, tag="e32")
                    nc.vector.tensor_tensor(
                        out=e32[:].rearrange("p (w d) -> p w d", d=D),
                        in0=acc[:].rearrange("p (w d) -> p w d", d=D),
                        in1=dv[:].unsqueeze(2).to_broadcast([P, NT, D]),
                        op=mybir.AluOpType.mult)
                    nc.sync.dma_start(
                        edges_o[k, :, :].rearrange("(w p) d -> p w d", p=P),
                        e32[:].rearrange("p (w d) -> p w d", d=D))
                col_off = spmm(k, 1, e_full, col_off)

                with tc.tile_pool(name=f"po{k}", bufs=3) as po, \
                     tc.tile_pool(name=f"poP{k}", bufs=2, space="PSUM") as poP:
                    # node = dinv * acc, folded in ahead of the softmax
                    a3 = acc[:].rearrange("p (w d) -> p w d", d=D)
                    nc.vector.tensor_tensor(
                        out=a3, in0=a3,
                        in1=dv[:].unsqueeze(2).to_broadcast([P, NT, D]),
                        op=mybir.AluOpType.mult)
                    nbv = 14
                    for w0 in range(0, NT, nbv):
                        bw = min(nbv, NT - w0)
                        sl = acc[:, w0 * D:(w0 + bw) * D]
                        sl3 = sl.rearrange("p (b d) -> p b d", d=D)
                        nc.scalar.activation(sl, sl, AF.Exp)
                        ssum = po.tile([P, nbv], F32, tag="ssum")
                        nc.vector.reduce_sum(ssum[:, 0:bw], sl3,
                                             axis=mybir.AxisListType.X)
                        nc.vector.reciprocal(ssum[:, 0:bw], ssum[:, 0:bw])
                        nc.vector.tensor_tensor(
                            out=sl3, in0=sl3,
                            in1=ssum[:, 0:bw].unsqueeze(2).to_broadcast(
                                [P, bw, D]),
                            op=mybir.AluOpType.mult)
                    hq = 0
                    for (co, cn) in ccols:
                        psT = poP.tile([D, CCH], F32, tag="T")
                        for j in range(0, cn, P):
                            pj = min(P, cn - j)
                            w = (co + j) // P
                            nc.tensor.transpose(
                                psT[:, j:j + pj],
                                acc[:, w * D:(w + 1) * D][0:pj, :],
                                ident[0:pj, 0:pj])
                        smT = po.tile([D, CCH], F32, tag="smT")
                        nc.scalar.activation(smT[:, 0:cn], psT[:, 0:cn],
                                             AF.Copy)
                        psN = poP.tile([D, CCH], F32, tag="N")
                        nc.tensor.matmul(psN[:, 0:cn], lhsT=wfc1[:, :],
                                         rhs=smT[:, 0:cn], start=True,
                                         stop=True)
                        nodeT = po.tile([D, CCH], F32, tag="nodeT")
                        nc.scalar.activation(nodeT[:, 0:cn], psN[:, 0:cn],
                                             AF.Copy)
                        xc_s = po.tile([D, CCH], F32, tag="xc")
                        nc.sync.dma_start(xc_s[:, 0:cn], xT_d[:, co:co + cn])
                        psG = poP.tile([D, CCH], F32, tag="T")
                        nc.tensor.matmul(psG[:, 0:cn], lhsT=wfus[:, :],
                                         rhs=xc_s[:, 0:cn], start=True,
                                         stop=True)
                        t1x = po.tile([D, CCH], F32, tag="t1x")
                        nc.scalar.activation(t1x[:, 0:cn], psG[:, 0:cn],
                                             AF.Tanh, bias=b1c[:, 0:1])
                        psA0 = poP.tile([1, CCH], F32, tag="A")
                        nc.tensor.matmul(psA0[:, 0:cn], lhsT=w2c[:, :],
                                         rhs=t1x[:, 0:cn], start=True,
                                         stop=True)
                        psG2 = poP.tile([D, CCH], F32, tag="N")
                        nc.tensor.matmul(psG2[:, 0:cn], lhsT=wfus[:, :],
                                         rhs=nodeT[:, 0:cn], start=True,
                                         stop=True)
                        t1n = po.tile([D, CCH], F32, tag="t1n")
                        nc.scalar.activation(t1n[:, 0:cn], psG2[:, 0:cn],
                                             AF.Tanh, bias=b1c[:, 0:1])
                        psA1 = poP.tile([1, CCH], F32, tag="A")
                        nc.tensor.matmul(psA1[:, 0:cn], lhsT=w2c[:, :],
                                         rhs=t1n[:, 0:cn], start=True,
                                         stop=True)
                        a1s = po.tile([1, CCH], F32, tag="a1s")
                        nc.scalar.activation(a1s[:, 0:cn], psA1[:, 0:cn],
                                             AF.Copy)
                        s0 = po.tile([1, CCH], F32, tag="s0")
                        nc.vector.tensor_tensor(out=s0[:, 0:cn],
                                                in0=psA0[:, 0:cn],
                                                in1=a1s[:, 0:cn],
                                                op=mybir.AluOpType.subtract)
                        nc.scalar.activation(s0[:, 0:cn], s0[:, 0:cn],
                                             AF.Sigmoid)
                        s0b = poP.tile([D, CCH], F32, tag="A")
                        nc.tensor.matmul(s0b[:, 0:cn], lhsT=ones1[:, :],
                                         rhs=s0[:, 0:cn], start=True,
                                         stop=True)
                        diff = po.tile([D, CCH], F32, tag="diff")
                        nc.vector.tensor_tensor(out=diff[:, 0:cn],
                                                in0=xc_s[:, 0:cn],
                                                in1=nodeT[:, 0:cn],
                                                op=mybir.AluOpType.subtract)
                        nc.vector.tensor_tensor(
                            out=diff[:, 0:cn], in0=diff[:, 0:cn],
                            in1=s0b[:, 0:cn],
                            op=mybir.AluOpType.mult)
                        xnew = po.tile([D, CCH], F32, tag="xnew")
                        nc.vector.tensor_tensor(out=xnew[:, 0:cn],
                                                in0=nodeT[:, 0:cn],
                                                in1=diff[:, 0:cn],
                                                op=mybir.AluOpType.add)
                        nc.sync.dma_start(xT_d[:, co:co + cn], xnew[:, 0:cn])
                        nc.sync.dma_start(nodesT_o[k, :, co:co + cn],
                                          xnew[:, 0:cn])
                        if k < K - 1:
                            h_chunk(po, poP, xnew, co, cn, h_bounce,
                                    dv_tiles[k + 1][0])
                            while (hq < cfg.NB
                                   and co + cn >= int(cfg.QRS[hq + 1])):
                                ag_q(h_bounce, h_full, hq)
                                hq += 1
    nc.compile()
    return nc


_CACHE = {}


def _plan_key(plan):
    return tuple(
        (tuple(map(tuple, pk["nt"])),
         tuple(tuple(tuple(ch) for ch in ob) for ob in pk["chunks"]))
        for pk in plan)


def _get_nc(cfg, plan):
    key = (cfg.NU, cfg.NI, cfg.K, cfg.E, cfg.CH, _plan_key(plan))
    if key not in _CACHE:
        _CACHE[key] = build(cfg, plan)
    return _CACHE[key]


def prepare(cfg, x, hgc1_bias, fc1_W, fus_l1_W, fus_l1_b, fus_l2_W, fus_l2_b,
            rows, cols):
    """Host prep: build (cached) module + per-core input maps."""
    x = np.asarray(x, np.float32)
    plan, per_core = prep(cfg, rows, cols)
    nc = _get_nc(cfg, plan)

    core_of, loc_of, _ = _perm_maps(cfg)
    in_maps = []
    for m in range(NCORES):
        xm = np.zeros((cfg.SHP, D), np.float32)
        sel = core_of == m
        xm[loc_of[sel]] = x[sel]
        in_maps.append({
            "xT": np.ascontiguousarray(xm.T),
            "biascol": np.asarray(hgc1_bias, np.float32).reshape(D, 1),
            "fc1_WT": np.ascontiguousarray(np.asarray(fc1_W, np.float32).T),
            "fus1_WT": np.ascontiguousarray(np.asarray(fus_l1_W, np.float32).T),
            "b1col": np.asarray(fus_l1_b, np.float32).reshape(D, 1),
            "w2col": np.ascontiguousarray(
                np.asarray(fus_l2_W, np.float32).reshape(1, D).T),
            "gidx": per_core[m]["gidx"],
            "sr": per_core[m]["sr"],
            "dinvw": per_core[m]["dinvw"],
            "dinv2w": per_core[m]["dinv2w"],
        })
    return nc, in_maps


def unshard(cfg, results):
    """Per-core outputs -> full (nodes, edges)."""
    core_of, loc_of, _ = _perm_maps(cfg)
    N = cfg.NU + cfg.NI
    nodes = np.zeros((cfg.K, N, D), np.float32)
    edges = np.zeros((cfg.K, N, D), np.float32)
    for m in range(NCORES):
        sel = core_of == m
        nodesT = np.asarray(results[m]["nodesT"]).reshape(cfg.K, D, cfg.SHP)
        edg = np.asarray(results[m]["edges"]).reshape(cfg.K, cfg.SHP, D)
        nodes[:, sel, :] = nodesT[:, :, loc_of[sel]].transpose(0, 2, 1)
        edges[:, sel, :] = edg[:, loc_of[sel], :]
    return nodes, edges


def run(cfg, x, hgc1_bias, fc1_W, fus_l1_W, fus_l1_b, fus_l2_W, fus_l2_b,
        rows, cols, sim=False):
    nc, in_maps = prepare(cfg, x, hgc1_bias, fc1_W, fus_l1_W, fus_l1_b,
                          fus_l2_W, fus_l2_b, rows, cols)

    if sim:
        from concourse import bass_interp
        simu = bass_interp.MultiCoreSim(nc, NCORES, require_finite=False,
                                        require_nnan=False)
        for m in range(NCORES):
            for kk, a in in_maps[m].items():
                simu.cores[m].tensor(kk)[:] = a
        simu.simulate()
        results = [{"nodesT": simu.cores[m].mem_tensor("nodesT"),
                    "edges": simu.cores[m].mem_tensor("edges")}
                   for m in range(NCORES)]
        exec_ns = None
    else:
        from concourse.bass_utils import run_bass_kernel_spmd
        res = run_bass_kernel_spmd(nc, in_maps, core_ids=list(range(NCORES)))
        results = res.results
        exec_ns = res.exec_time_ns

    return unshard(cfg, results), exec_ns


def kernel(x, hgc1_bias, fc1_W, fus_l1_W, fus_l1_b, fus_l2_W, fus_l2_b,
           rows, cols):
    cfg = Cfg(NU=50000, NI=50000, K=4, E=1000000, CH=6144)
    (nodes, edges), _ = run(cfg, x, hgc1_bias, fc1_W, fus_l1_W, fus_l1_b,
                            fus_l2_W, fus_l2_b, rows, cols)
    return nodes, edges
